# revision 11
# baseline (speedup 1.0000x reference)
"""Trainium2 Bass kernel for the GWFFN spiking-CNN block (nn_GWFFN).

Reference computation (multi-step LIF scan over T=4, eval-mode BN):
  up:   LIF -> 1x1 conv (128->512) -> BN
  conv: LIF -> grouped 3x3 conv (512->512, groups=8, pad=1) -> BN -> +h1
  down: LIF -> 1x1 conv (512->128) -> BN -> +x

Sharding: data-parallel over batch N=16 -> 8 cores x 2 samples. Weights are
replicated, no collectives; each core runs an identical program on its slice.

Per-core layout: channels on SBUF partitions (the 512-wide inner dim is 4
tiles of 128), free dim = (n_loc=2, h=32, w=32) = 2048 positions, processed
one time-step at a time. With v_t = w_t/2 the LIF scan becomes
    w_t = 0.5*u_{t-1} + x_t ;  s_t = (w_t >= 2) ;  u_t = (w_t < 2) * w_t
which is 3 fused vector ops per step (scalar_tensor_tensor / tensor_scalar).

Matmuls run in bf16 (spikes are exactly 0/1 in bf16; BN scale is folded into
the weights on the host). The grouped 3x3 conv is 9 shifted 1x1 matmuls
accumulating in PSUM over a zero-padded [2,34,36] spatial SBUF layout; the
8 groups (64 channels each) are packed as 4 concurrent 64x64 tile_position
matmuls so the full 128x128 PE array is used. The +h1 residual is preloaded
into PSUM with identity / half-swap permutation matmuls (the anti-diagonal
array tiles emit their two groups partition-swapped; the swap is undone by
permuting the down-projection weight rows on the host).
"""

import numpy as np
import ml_dtypes

import concourse.bacc as bacc
import concourse.mybir as mybir
import concourse.tile as tile
from concourse.bass_utils import run_bass_kernel_spmd

FP32 = mybir.dt.float32
BF16 = mybir.dt.bfloat16
ALU = mybir.AluOpType
ACTF = mybir.ActivationFunctionType
BF = ml_dtypes.bfloat16

T, NFULL, C, H, W = 4, 16, 128, 32, 32
INNER, GROUPS = 512, 8
GSZ = INNER // GROUPS   # 64
NCORES = 8
NLOC = NFULL // NCORES  # 2
HW = H * W              # 1024
F = NLOC * HW           # 2048 free positions per time-step
CH = 512                # matmul free-dim chunk (one PSUM bank fp32)
HP, WP = H + 2, W + 4   # padded spatial (W padded by 2 each side: 4B align)
EPS = 1e-5

_CACHE = {}


def _mm(nc, out, lhsT, rhs, start, stop):
    nc.tensor.matmul(out, lhsT, rhs, start=start, stop=stop,
                     skip_group_check=True)


def _build_nc():
    nc = bacc.Bacc("TRN2", target_bir_lowering=False)

    x_d = nc.dram_tensor("x", [T, NLOC, C, H, W], FP32, kind="ExternalInput")
    wup_d = nc.dram_tensor("wupT", [C, INNER], BF16, kind="ExternalInput")
    wcv_d = nc.dram_tensor("wconvP", [128, 2, 9, 128], BF16, kind="ExternalInput")
    wdn_d = nc.dram_tensor("wdnT", [128, 4, 128], BF16, kind="ExternalInput")
    jm_d = nc.dram_tensor("jmat", [128, 2, 128], BF16, kind="ExternalInput")
    o_d = nc.dram_tensor("out", [T, NLOC, C, H, W], FP32, kind="ExternalOutput")

    with tile.TileContext(nc) as tc:
        with (
            tc.tile_pool(name="const", bufs=1) as cpool,
            tc.tile_pool(name="state", bufs=1) as spool,
            tc.tile_pool(name="work", bufs=2) as wpool,
            tc.tile_pool(name="psum", bufs=4, space="PSUM") as ppool,
        ):
            # ---- constants -------------------------------------------------
            wup_sb = cpool.tile([C, INNER], BF16)
            nc.sync.dma_start(out=wup_sb[:], in_=wup_d[:])
            wcv_sb = cpool.tile([128, 2, 9, 128], BF16)
            nc.sync.dma_start(out=wcv_sb[:], in_=wcv_d[:])
            wdn_sb = cpool.tile([128, 4, 128], BF16)
            nc.sync.dma_start(out=wdn_sb[:], in_=wdn_d[:])
            jm_sb = cpool.tile([128, 2, 128], BF16)
            nc.sync.dma_start(out=jm_sb[:], in_=jm_d[:])

            # ---- persistent LIF state + padded spike buffers ---------------
            u1 = spool.tile([128, F], FP32)
            nc.gpsimd.memset(u1[:], 0.0)
            u2, u3, s2p = [], [], []
            for i in range(4):
                u2t = spool.tile([128, F], BF16, name=f"u2_{i}")
                nc.gpsimd.memset(u2t[:], 0.0)
                u2.append(u2t)
                u3t = spool.tile([128, F], BF16, name=f"u3_{i}")
                nc.gpsimd.memset(u3t[:], 0.0)
                u3.append(u3t)
                s2t = spool.tile([128, NLOC, HP, WP], BF16, name=f"s2p_{i}")
                nc.gpsimd.memset(s2t[:], 0.0)
                s2p.append(s2t)

            # down-stage tensors carried from step t-1 (emitted after conv(t)
            # so the PE never stalls waiting on the LIF3 chain)
            pend = None

            def emit_down(t, s3, x_sb):
                out_sb = wpool.tile([128, F], FP32, tag="osb", bufs=2,
                                    name=f"osb_{t}")
                for p in range(2):            # pair p <-> n=p, halves h0=0,16
                    ps_dn = ppool.tile([128, 2 * CH], FP32, tag="ps",
                                       name=f"psdn_{t}_{p}")
                    for hh in range(2):
                        o_sl = ps_dn[:, hh * CH:(hh + 1) * CH]
                        c = 2 * p + hh
                        for kt in range(4):
                            _mm(nc, o_sl, wdn_sb[:, kt, :],
                                s3[kt][:, c * CH:(c + 1) * CH],
                                start=(kt == 0), stop=(kt == 3))
                    nc.vector.tensor_tensor(
                        out=out_sb[:, p * HW:(p + 1) * HW],
                        in0=ps_dn[:],
                        in1=x_sb[:, p * HW:(p + 1) * HW],
                        op=ALU.add)
                    if p == 1:
                        nc.sync.dma_start(
                            out=o_d[t].rearrange("n c h w -> c n h w"),
                            in_=out_sb.rearrange("p (n h w) -> p n h w",
                                                 n=NLOC, h=H))

            for t in range(T):
                # ---- load x_t --------------------------------------------
                x_sb = wpool.tile([128, F], FP32, tag="x", bufs=2,
                                  name=f"x_{t}")
                nc.sync.dma_start(
                    out=x_sb.rearrange("p (n h w) -> p n h w", n=NLOC, h=H),
                    in_=x_d[t].rearrange("n c h w -> c n h w"))

                # ---- LIF1 (fp32) -----------------------------------------
                if t == 0:
                    w1 = x_sb
                else:
                    w1 = wpool.tile([128, F], FP32, tag="w1", bufs=1,
                                    name=f"w1_{t}")
                    nc.vector.scalar_tensor_tensor(
                        out=w1[:], in0=u1[:], scalar=0.5, in1=x_sb[:],
                        op0=ALU.mult, op1=ALU.add)
                s1 = wpool.tile([128, F], BF16, tag="s1", bufs=2,
                                name=f"s1_{t}")
                nc.gpsimd.tensor_scalar(
                    out=s1[:], in0=w1[:], scalar1=2.0, scalar2=None,
                    op0=ALU.is_ge)
                nc.vector.scalar_tensor_tensor(
                    out=u1[:], in0=w1[:], scalar=2.0, in1=w1[:],
                    op0=ALU.is_lt, op1=ALU.mult)

                # ---- up 1x1 matmul + evac + LIF2 -------------------------
                h1, w2 = [], []
                for ct in range(4):
                    h1t = wpool.tile([128, F], BF16, tag="hbuf", bufs=7,
                                     name=f"h1_{t}_{ct}")
                    for p in range(2):
                        ps_up = ppool.tile([128, 2 * CH], FP32, tag="ps",
                                           name=f"psup_{t}_{ct}_{p}")
                        for hh in range(2):
                            c = 2 * p + hh
                            _mm(nc, ps_up[:, hh * CH:(hh + 1) * CH],
                                wup_sb[:, 128 * ct:128 * (ct + 1)],
                                s1[:, c * CH:(c + 1) * CH],
                                start=True, stop=True)
                        nc.scalar.copy(
                            out=h1t[:, p * HW:(p + 1) * HW], in_=ps_up[:])
                    h1.append(h1t)

                    w2t = wpool.tile([128, F], BF16, tag="wbuf", bufs=6,
                                     name=f"w2_{t}_{ct}")
                    nc.vector.scalar_tensor_tensor(
                        out=w2t[:], in0=u2[ct][:], scalar=0.5, in1=h1t[:],
                        op0=ALU.mult, op1=ALU.add)
                    w2.append(w2t)
                    nc.gpsimd.tensor_scalar(
                        out=s2p[ct][:, :, 1:1 + H, 2:2 + W],
                        in0=w2t.rearrange("p (n h w) -> p n h w", n=NLOC, h=H),
                        scalar1=2.0, scalar2=None, op0=ALU.is_ge)
                    # Pool has no scalar_tensor_tensor: mask then multiply
                    m2t = wpool.tile([128, F], BF16, tag="m2", bufs=2,
                                     name=f"m2_{t}_{ct}")
                    nc.gpsimd.tensor_scalar(
                        out=m2t[:], in0=w2t[:], scalar1=2.0, scalar2=None,
                        op0=ALU.is_lt)
                    nc.gpsimd.tensor_tensor(
                        out=u2[ct][:], in0=w2t[:], in1=m2t[:], op=ALU.mult)

                # ---- grouped 3x3 conv (+h1 preload) + evac ---------------
                h2 = [None] * 4
                for q in range(2):
                    ta, tb = 2 * q, 2 * q + 1   # s2 tiles feeding this quad
                    h2a = wpool.tile([128, F], BF16, tag="hbuf", bufs=7,
                                     name=f"h2_{t}_{ta}")
                    h2b = wpool.tile([128, F], BF16, tag="hbuf", bufs=7,
                                     name=f"h2_{t}_{tb}")
                    h2[ta], h2[tb] = h2a, h2b
                    for p in range(2):
                        P1 = ppool.tile([128, 2 * CH], FP32, tag="ps",
                                        name=f"psc1_{t}_{q}_{p}")
                        P2 = ppool.tile([128, 2 * CH], FP32, tag="ps",
                                        name=f"psc2_{t}_{q}_{p}")
                        for hh in range(2):
                            c = 2 * p + hh
                            sl = slice(c * CH, (c + 1) * CH)
                            _mm(nc, P1[:, hh * CH:(hh + 1) * CH],
                                jm_sb[:, 0, :], h1[ta][:, sl],
                                start=True, stop=False)
                            _mm(nc, P2[:, hh * CH:(hh + 1) * CH],
                                jm_sb[:, 1, :], h1[tb][:, sl],
                                start=True, stop=False)
                        for tap in range(9):
                            dy, dx = tap // 3, tap % 3
                            last = tap == 8
                            wq = wcv_sb[:, q, tap, :]
                            for hh in range(2):
                                h0 = 16 * hh
                                osl = slice(hh * CH, (hh + 1) * CH)
                                ra = s2p[ta][:, p, h0 + dy:h0 + dy + 16,
                                             1 + dx:33 + dx]
                                rb = s2p[tb][:, p, h0 + dy:h0 + dy + 16,
                                             1 + dx:33 + dx]
                                # T1: group 4q   rows 0-63  -> P1[0:64]
                                _mm(nc, P1[0:64, osl], wq[0:64, 0:64],
                                    ra[0:64], start=False, stop=last)
                                # T2: group 4q+1 rows 64-127 -> P1[64:128]
                                _mm(nc, P1[64:128, osl], wq[64:128, 64:128],
                                    ra[64:128], start=False, stop=last)
                                # T3: group 4q+2 rows 0-63  -> P2[64:128]
                                _mm(nc, P2[64:128, osl], wq[0:64, 64:128],
                                    rb[0:64], start=False, stop=last)
                                # T4: group 4q+3 rows 64-127 -> P2[0:64]
                                _mm(nc, P2[0:64, osl], wq[64:128, 0:64],
                                    rb[64:128], start=False, stop=last)
                        nc.scalar.copy(out=h2a[:, p * HW:(p + 1) * HW],
                                       in_=P1[:])
                        nc.scalar.copy(out=h2b[:, p * HW:(p + 1) * HW],
                                       in_=P2[:])

                # ---- LIF3 ------------------------------------------------
                s3 = []
                for ct in range(4):
                    w3t = wpool.tile([128, F], BF16, tag="wbuf", bufs=6,
                                     name=f"w3_{t}_{ct}")
                    nc.vector.scalar_tensor_tensor(
                        out=w3t[:], in0=u3[ct][:], scalar=0.5, in1=h2[ct][:],
                        op0=ALU.mult, op1=ALU.add)
                    s3t = wpool.tile([128, F], BF16, tag="s3", bufs=7,
                                     name=f"s3_{t}_{ct}")
                    nc.gpsimd.tensor_scalar(
                        out=s3t[:], in0=w3t[:], scalar1=2.0, scalar2=None,
                        op0=ALU.is_ge)
                    s3.append(s3t)
                    nc.vector.scalar_tensor_tensor(
                        out=u3[ct][:], in0=w3t[:], scalar=2.0, in1=w3t[:],
                        op0=ALU.is_lt, op1=ALU.mult)

                # ---- down stage of the previous step ---------------------
                if pend is not None:
                    emit_down(*pend)
                pend = (t, s3, x_sb)

            emit_down(*pend)

    nc.compile()
    return nc


def _prep_weights(inputs):
    """Fold BN into weights, pack/permute for the on-chip layout (bf16)."""
    f32 = np.float32
    sc_up = (inputs["g_up"] / np.sqrt(inputs["v_up"] + EPS)).astype(f32)
    sc_cv = (inputs["g_conv"] / np.sqrt(inputs["v_conv"] + EPS)).astype(f32)
    sc_dn = (inputs["g_down"] / np.sqrt(inputs["v_down"] + EPS)).astype(f32)
    for nm, sc, g in (("up", sc_up, "g_up"), ("conv", sc_cv, "g_conv"),
                      ("down", sc_dn, "g_down")):
        shift = inputs[f"b_{nm}"] - inputs[f"m_{nm}"] * sc
        if np.abs(shift).max() > 0:
            raise NotImplementedError("nonzero BN shift not supported")

    w_up = np.asarray(inputs["w_up"], f32)[:, :, 0, 0] * sc_up[:, None]
    wupT = np.ascontiguousarray(w_up.T).astype(BF)             # [128, 512]

    w_cv = np.asarray(inputs["w_conv"], f32) * sc_cv[:, None, None, None]
    wcvP = np.zeros((128, 2, 9, 128), f32)
    for q in range(2):
        for tap in range(9):
            dy, dx = tap // 3, tap % 3
            # W_g[ci, co] = w_conv_eff[64g + co, ci, dy, dx]
            def blk(g):
                return np.ascontiguousarray(
                    w_cv[64 * g:64 * (g + 1), :, dy, dx].T)
            wcvP[0:64, q, tap, 0:64] = blk(4 * q)
            wcvP[64:128, q, tap, 64:128] = blk(4 * q + 1)
            wcvP[0:64, q, tap, 64:128] = blk(4 * q + 2)
            wcvP[64:128, q, tap, 0:64] = blk(4 * q + 3)
    wcvP = wcvP.astype(BF)

    w_dn = np.asarray(inputs["w_down"], f32)[:, :, 0, 0] * sc_dn[:, None]
    # s3 tile layouts: kt even natural, kt odd half-swapped ([g3|g2] etc.)
    wdnT = np.zeros((128, 4, 128), f32)
    for kt in range(4):
        rows = np.arange(128) + 128 * kt
        if kt % 2 == 1:
            rows = np.concatenate([rows[64:], rows[:64]])
        wdnT[:, kt, :] = w_dn[:, rows].T
    wdnT = wdnT.astype(BF)

    jm = np.zeros((128, 2, 128), f32)
    jm[:, 0, :] = np.eye(128)
    jm[np.arange(128), 1, (np.arange(128) + 64) % 128] = 1.0
    jm = jm.astype(BF)
    return wupT, wcvP, wdnT, jm


def run(inputs, trace=False):
    if "nc" not in _CACHE:
        _CACHE["nc"] = _build_nc()
    nc = _CACHE["nc"]

    wupT, wcvP, wdnT, jm = _prep_weights(inputs)
    x = np.asarray(inputs["x"], np.float32)
    in_maps = []
    for i in range(NCORES):
        in_maps.append({
            "x": np.ascontiguousarray(x[:, NLOC * i:NLOC * (i + 1)]),
            "wupT": wupT, "wconvP": wcvP, "wdnT": wdnT, "jmat": jm,
        })
    res = run_bass_kernel_spmd(nc, in_maps, core_ids=list(range(NCORES)),
                               trace=trace)
    out = np.concatenate([r["out"] for r in res.results], axis=1)
    return out, res


def kernel(**inputs):
    out, _ = run(inputs, trace=False)
    return out


# revision 13
# speedup vs baseline: 7.9691x; 7.9691x over previous
"""Trainium2 Bass kernel for the GWFFN spiking-CNN block (nn_GWFFN).

Reference computation (multi-step LIF scan over T=4, eval-mode BN):
  up:   LIF -> 1x1 conv (128->512) -> BN
  conv: LIF -> grouped 3x3 conv (512->512, groups=8, pad=1) -> BN -> +h1
  down: LIF -> 1x1 conv (512->128) -> BN -> +x

Sharding: data-parallel over batch N=16 -> 8 cores x 2 samples. Weights are
replicated, no collectives; each core runs an identical program on its slice.

Per-core layout: channels on SBUF partitions (the 512-wide inner dim is 4
tiles of 128), free dim = (n_loc=2, h=32, w=32) = 2048 positions, one
time-step at a time. Scaling the LIF state by 2 (v_t = w_t/2, u' = v after
reset) turns the scan into
    w_t = u'_{t-1} + x_t ;  s_t = (w_t >= 2) ;  u'_t = w_t * m_t
with m_t = 0.5*(w_t < 2). The matmuls consume m instead of s via
s = 1 - 2m: weights are scaled by -2 and the constant W@1 term folds into a
per-channel bias (applied for free in the ACT-engine PSUM->SBUF evacuation;
the grouped conv's zero-pad border stores m=0.5 so the ones-field is exact
at the edges too). This keeps every elementwise op on the fast
tensor_tensor / dual-op tensor_scalar paths (the TensorScalarPtr and
Pool-engine comparison ops measure 10-30x slower).

Matmuls run in bf16 (m is exactly {0, 0.5} in bf16; BN scale folded into
weights on the host). The grouped 3x3 conv is 9 shifted 1x1 matmuls
accumulating in PSUM over the padded [2,34,36] layout, packed as 4
concurrent 64x64 tile_position matmuls (4 groups at once); the +h1 residual
is preloaded into PSUM with identity / half-swap permutation matmuls (the
anti-diagonal array tiles emit their groups partition-swapped; the swap is
undone by permuting the down-projection weight rows on the host). The
down-projection bias is added with a K=2 rank-2 matmul of a bf16 hi/lo
split of the bias against a ones vector.
"""

import numpy as np
import ml_dtypes

import concourse.bacc as bacc
import concourse.mybir as mybir
import concourse.tile as tile
from concourse.bass_utils import run_bass_kernel_spmd

FP32 = mybir.dt.float32
BF16 = mybir.dt.bfloat16
ALU = mybir.AluOpType
ACTF = mybir.ActivationFunctionType
BF = ml_dtypes.bfloat16

T, NFULL, C, H, W = 4, 16, 128, 32, 32
INNER, GROUPS = 512, 8
NCORES = 8
NLOC = NFULL // NCORES  # 2
HW = H * W              # 1024
F = NLOC * HW           # 2048 free positions per time-step
CH = 512                # matmul free-dim chunk (one PSUM bank fp32)
HP, WP = H + 2, W + 4   # padded spatial (W padded by 2 each side: 4B align)
EPS = 1e-5

_CACHE = {}


def _mm(nc, out, lhsT, rhs, start, stop):
    nc.tensor.matmul(out, lhsT, rhs, start=start, stop=stop,
                     skip_group_check=True)


def _build_nc():
    nc = bacc.Bacc("TRN2", target_bir_lowering=False)

    x_d = nc.dram_tensor("x", [T, NLOC, C, H, W], FP32, kind="ExternalInput")
    wup_d = nc.dram_tensor("wupT", [C, INNER], BF16, kind="ExternalInput")
    wcv_d = nc.dram_tensor("wconvP", [128, 2, 9, 128], BF16, kind="ExternalInput")
    wdn_d = nc.dram_tensor("wdnT", [128, 4, 128], BF16, kind="ExternalInput")
    jm_d = nc.dram_tensor("jmat", [128, 2, 128], BF16, kind="ExternalInput")
    bia_d = nc.dram_tensor("bias", [128, 8], FP32, kind="ExternalInput")
    bdn_d = nc.dram_tensor("biasdn2", [2, 128], BF16, kind="ExternalInput")
    o_d = nc.dram_tensor("out", [T, NLOC, C, H, W], FP32, kind="ExternalOutput")

    with tile.TileContext(nc) as tc:
        with (
            tc.tile_pool(name="const", bufs=1) as cpool,
            tc.tile_pool(name="state", bufs=1) as spool,
            tc.tile_pool(name="work", bufs=2) as wpool,
            tc.tile_pool(name="psum", bufs=4, space="PSUM") as ppool,
        ):
            # ---- constants -------------------------------------------------
            wup_sb = cpool.tile([C, INNER], BF16)
            nc.sync.dma_start(out=wup_sb[:], in_=wup_d[:])
            wcv_sb = cpool.tile([128, 2, 9, 128], BF16)
            nc.sync.dma_start(out=wcv_sb[:], in_=wcv_d[:])
            wdn_sb = cpool.tile([128, 4, 128], BF16)
            nc.sync.dma_start(out=wdn_sb[:], in_=wdn_d[:])
            jm_sb = cpool.tile([128, 2, 128], BF16)
            nc.sync.dma_start(out=jm_sb[:], in_=jm_d[:])
            bia_sb = cpool.tile([128, 8], FP32)
            nc.sync.dma_start(out=bia_sb[:], in_=bia_d[:])
            bdn_sb = cpool.tile([2, 128], BF16)
            nc.sync.dma_start(out=bdn_sb[:], in_=bdn_d[:])
            ones_sb = cpool.tile([2, CH], BF16)
            nc.gpsimd.memset(ones_sb[:], 1.0)

            # ---- persistent LIF state + padded m2 buffers ------------------
            u1 = spool.tile([128, F], FP32)
            nc.gpsimd.memset(u1[:], 0.0)
            u2, u3, s2p = [], [], []
            for i in range(4):
                u2t = spool.tile([128, F], BF16, name=f"u2_{i}")
                nc.gpsimd.memset(u2t[:], 0.0)
                u2.append(u2t)
                u3t = spool.tile([128, F], BF16, name=f"u3_{i}")
                nc.gpsimd.memset(u3t[:], 0.0)
                u3.append(u3t)
                s2t = spool.tile([128, NLOC, HP, WP], BF16, name=f"s2p_{i}")
                # border value 0.5 == "no spike" for the 1-2m encoding
                nc.gpsimd.memset(s2t[:], 0.5)
                s2p.append(s2t)

            # down-stage of step t-1, emitted after conv(t) so the PE
            # stream never waits on the t-1 LIF3 chain
            pend = None

            def emit_down(t, m3, x_sb):
                out_sb = wpool.tile([128, F], FP32, tag="osb", bufs=2,
                                    name=f"osb_{t}")
                for p in range(2):            # pair p <-> n=p
                    ps_dn = ppool.tile([128, 2 * CH], FP32, tag="ps",
                                       name=f"psdn_{t}_{p}")
                    for hh in range(2):
                        o_sl = ps_dn[:, hh * CH:(hh + 1) * CH]
                        c = 2 * p + hh
                        for kt in range(4):
                            _mm(nc, o_sl, wdn_sb[:, kt, :],
                                m3[kt][:, c * CH:(c + 1) * CH],
                                start=(kt == 0), stop=False)
                        # + bias_dn via rank-2 ones matmul (bf16 hi+lo)
                        _mm(nc, o_sl, bdn_sb[:], ones_sb[:],
                            start=False, stop=True)
                    nc.vector.tensor_tensor(
                        out=out_sb[:, p * HW:(p + 1) * HW],
                        in0=ps_dn[:],
                        in1=x_sb[:, p * HW:(p + 1) * HW],
                        op=ALU.add)
                nc.sync.dma_start(
                    out=o_d[t].rearrange("n c h w -> c n h w"),
                    in_=out_sb.rearrange("p (n h w) -> p n h w", n=NLOC, h=H))

            for t in range(T):
                # ---- load x_t --------------------------------------------
                x_sb = wpool.tile([128, F], FP32, tag="x", bufs=2,
                                  name=f"x_{t}")
                nc.sync.dma_start(
                    out=x_sb.rearrange("p (n h w) -> p n h w", n=NLOC, h=H),
                    in_=x_d[t].rearrange("n c h w -> c n h w"))

                # ---- LIF1 (fp32) -----------------------------------------
                if t == 0:
                    w1 = x_sb
                else:
                    w1 = wpool.tile([128, F], FP32, tag="w1", bufs=1,
                                    name=f"w1_{t}")
                    nc.vector.tensor_tensor(
                        out=w1[:], in0=u1[:], in1=x_sb[:], op=ALU.add)
                m1 = wpool.tile([128, F], BF16, tag="m1", bufs=2,
                                name=f"m1_{t}")
                nc.vector.tensor_scalar(
                    out=m1[:], in0=w1[:], scalar1=2.0, scalar2=0.5,
                    op0=ALU.is_lt, op1=ALU.mult)
                nc.vector.tensor_tensor(
                    out=u1[:], in0=w1[:], in1=m1[:], op=ALU.mult)

                # ---- up 1x1 matmul + evac(+bias) + LIF2 ------------------
                h1, w2 = [], []
                for ct in range(4):
                    h1t = wpool.tile([128, F], BF16, tag="hbuf", bufs=7,
                                     name=f"h1_{t}_{ct}")
                    for p in range(2):
                        ps_up = ppool.tile([128, 2 * CH], FP32, tag="ps",
                                           name=f"psup_{t}_{ct}_{p}")
                        for hh in range(2):
                            c = 2 * p + hh
                            _mm(nc, ps_up[:, hh * CH:(hh + 1) * CH],
                                wup_sb[:, 128 * ct:128 * (ct + 1)],
                                m1[:, c * CH:(c + 1) * CH],
                                start=True, stop=True)
                        nc.scalar.activation(
                            out=h1t[:, p * HW:(p + 1) * HW], in_=ps_up[:],
                            func=ACTF.Identity, bias=bia_sb[:, ct:ct + 1],
                            scale=1.0)
                    h1.append(h1t)

                    if t == 0:
                        w2t = h1t
                    else:
                        w2t = wpool.tile([128, F], BF16, tag="wbuf", bufs=6,
                                         name=f"w2_{t}_{ct}")
                        nc.vector.tensor_tensor(
                            out=w2t[:], in0=u2[ct][:], in1=h1t[:], op=ALU.add)
                    w2.append(w2t)
                    nc.vector.tensor_scalar(
                        out=s2p[ct][:, :, 1:1 + H, 2:2 + W],
                        in0=w2t.rearrange("p (n h w) -> p n h w", n=NLOC, h=H),
                        scalar1=2.0, scalar2=0.5, op0=ALU.is_lt, op1=ALU.mult)
                    nc.gpsimd.tensor_tensor(
                        out=u2[ct].rearrange("p (n h w) -> p n h w",
                                             n=NLOC, h=H),
                        in0=w2t.rearrange("p (n h w) -> p n h w", n=NLOC, h=H),
                        in1=s2p[ct][:, :, 1:1 + H, 2:2 + W],
                        op=ALU.mult)

                # ---- grouped 3x3 conv (+h1 preload) + evac(+bias) --------
                h2 = [None] * 4
                for q in range(2):
                    ta, tb = 2 * q, 2 * q + 1   # s2 tiles feeding this quad
                    h2a = wpool.tile([128, F], BF16, tag="hbuf", bufs=7,
                                     name=f"h2_{t}_{ta}")
                    h2b = wpool.tile([128, F], BF16, tag="hbuf", bufs=7,
                                     name=f"h2_{t}_{tb}")
                    h2[ta], h2[tb] = h2a, h2b
                    for p in range(2):
                        P1 = ppool.tile([128, 2 * CH], FP32, tag="ps",
                                        name=f"psc1_{t}_{q}_{p}")
                        P2 = ppool.tile([128, 2 * CH], FP32, tag="ps",
                                        name=f"psc2_{t}_{q}_{p}")
                        for hh in range(2):
                            c = 2 * p + hh
                            sl = slice(c * CH, (c + 1) * CH)
                            _mm(nc, P1[:, hh * CH:(hh + 1) * CH],
                                jm_sb[:, 0, :], h1[ta][:, sl],
                                start=True, stop=False)
                            _mm(nc, P2[:, hh * CH:(hh + 1) * CH],
                                jm_sb[:, 1, :], h1[tb][:, sl],
                                start=True, stop=False)
                        for tap in range(9):
                            dy, dx = tap // 3, tap % 3
                            last = tap == 8
                            wq = wcv_sb[:, q, tap, :]
                            for hh in range(2):
                                h0 = 16 * hh
                                osl = slice(hh * CH, (hh + 1) * CH)
                                ra = s2p[ta][:, p, h0 + dy:h0 + dy + 16,
                                             1 + dx:33 + dx]
                                rb = s2p[tb][:, p, h0 + dy:h0 + dy + 16,
                                             1 + dx:33 + dx]
                                # T1: group 4q   rows 0-63  -> P1[0:64]
                                _mm(nc, P1[0:64, osl], wq[0:64, 0:64],
                                    ra[0:64], start=False, stop=last)
                                # T2: group 4q+1 rows 64-127 -> P1[64:128]
                                _mm(nc, P1[64:128, osl], wq[64:128, 64:128],
                                    ra[64:128], start=False, stop=last)
                                # T3: group 4q+2 rows 0-63  -> P2[64:128]
                                _mm(nc, P2[64:128, osl], wq[0:64, 64:128],
                                    rb[0:64], start=False, stop=last)
                                # T4: group 4q+3 rows 64-127 -> P2[0:64]
                                _mm(nc, P2[0:64, osl], wq[64:128, 0:64],
                                    rb[64:128], start=False, stop=last)
                        nc.scalar.activation(
                            out=h2a[:, p * HW:(p + 1) * HW], in_=P1[:],
                            func=ACTF.Identity, bias=bia_sb[:, 4 + ta:5 + ta],
                            scale=1.0)
                        nc.scalar.activation(
                            out=h2b[:, p * HW:(p + 1) * HW], in_=P2[:],
                            func=ACTF.Identity, bias=bia_sb[:, 4 + tb:5 + tb],
                            scale=1.0)

                # ---- LIF3 ------------------------------------------------
                m3 = []
                for ct in range(4):
                    if t == 0:
                        w3t = h2[ct]
                    else:
                        w3t = wpool.tile([128, F], BF16, tag="wbuf", bufs=6,
                                         name=f"w3_{t}_{ct}")
                        nc.vector.tensor_tensor(
                            out=w3t[:], in0=u3[ct][:], in1=h2[ct][:],
                            op=ALU.add)
                    m3t = wpool.tile([128, F], BF16, tag="m3", bufs=7,
                                     name=f"m3_{t}_{ct}")
                    nc.vector.tensor_scalar(
                        out=m3t[:], in0=w3t[:], scalar1=2.0, scalar2=0.5,
                        op0=ALU.is_lt, op1=ALU.mult)
                    m3.append(m3t)
                    nc.vector.tensor_tensor(
                        out=u3[ct][:], in0=w3t[:], in1=m3t[:], op=ALU.mult)

                # ---- down stage of the previous step ---------------------
                if pend is not None:
                    emit_down(*pend)
                pend = (t, m3, x_sb)

            emit_down(*pend)

    nc.compile()
    return nc


def _prep_weights(inputs):
    """Fold BN into weights, apply the s = 1-2m encoding (scale by -2 and
    compute per-channel ones-biases), pack/permute for the on-chip layout."""
    f32 = np.float32
    sc_up = (inputs["g_up"] / np.sqrt(inputs["v_up"] + EPS)).astype(f32)
    sc_cv = (inputs["g_conv"] / np.sqrt(inputs["v_conv"] + EPS)).astype(f32)
    sc_dn = (inputs["g_down"] / np.sqrt(inputs["v_down"] + EPS)).astype(f32)
    shifts = []
    for nm, sc in (("up", sc_up), ("conv", sc_cv), ("down", sc_dn)):
        shifts.append(inputs[f"b_{nm}"] - inputs[f"m_{nm}"] * sc)
    if max(np.abs(s).max() for s in shifts) > 0:
        raise NotImplementedError("nonzero BN shift not supported")

    w_up = np.asarray(inputs["w_up"], f32)[:, :, 0, 0] * sc_up[:, None]
    wupT = np.ascontiguousarray((-2.0 * w_up).T).astype(BF)    # [128, 512]
    bias_up = w_up.sum(axis=1)                                 # [512]

    w_cv = np.asarray(inputs["w_conv"], f32) * sc_cv[:, None, None, None]
    wcvP = np.zeros((128, 2, 9, 128), f32)
    for q in range(2):
        for tap in range(9):
            dy, dx = tap // 3, tap % 3

            def blk(g):
                # W_g[ci, co] = -2 * w_conv_eff[64g + co, ci, dy, dx]
                return np.ascontiguousarray(
                    -2.0 * w_cv[64 * g:64 * (g + 1), :, dy, dx].T)
            wcvP[0:64, q, tap, 0:64] = blk(4 * q)
            wcvP[64:128, q, tap, 64:128] = blk(4 * q + 1)
            wcvP[0:64, q, tap, 64:128] = blk(4 * q + 2)
            wcvP[64:128, q, tap, 0:64] = blk(4 * q + 3)
    wcvP = wcvP.astype(BF)
    bias_cv = w_cv.sum(axis=(1, 2, 3))                         # [512]

    w_dn = np.asarray(inputs["w_down"], f32)[:, :, 0, 0] * sc_dn[:, None]
    bias_dn = w_dn.sum(axis=1)                                 # [128]
    # s3/m3 tile layouts: kt even natural, kt odd half-swapped ([g3|g2]...)
    wdnT = np.zeros((128, 4, 128), f32)
    for kt in range(4):
        rows = np.arange(128) + 128 * kt
        if kt % 2 == 1:
            rows = np.concatenate([rows[64:], rows[:64]])
        wdnT[:, kt, :] = -2.0 * w_dn[:, rows].T
    wdnT = wdnT.astype(BF)

    jm = np.zeros((128, 2, 128), f32)
    jm[:, 0, :] = np.eye(128)
    jm[np.arange(128), 1, (np.arange(128) + 64) % 128] = 1.0
    jm = jm.astype(BF)

    # bias tile [128, 8]: cols 0-3 = up bias per tile; 4-7 = conv bias per
    # conv-out tile (odd tiles half-swapped to match the P2 psum layout)
    bias = np.zeros((128, 8), f32)
    for ct in range(4):
        bias[:, ct] = bias_up[128 * ct:128 * (ct + 1)]
        bc = bias_cv[128 * ct:128 * (ct + 1)]
        if ct % 2 == 1:
            bc = np.concatenate([bc[64:], bc[:64]])
        bias[:, 4 + ct] = bc
    # down bias as bf16 hi + lo rows against a ones vector
    bdn_hi = bias_dn.astype(BF)
    bdn_lo = (bias_dn - bdn_hi.astype(f32)).astype(BF)
    bdn2 = np.stack([bdn_hi, bdn_lo], axis=0)                  # [2, 128]

    return wupT, wcvP, wdnT, jm, bias, bdn2


def run(inputs, trace=False):
    if "nc" not in _CACHE:
        _CACHE["nc"] = _build_nc()
    nc = _CACHE["nc"]

    wupT, wcvP, wdnT, jm, bias, bdn2 = _prep_weights(inputs)
    x = np.asarray(inputs["x"], np.float32)
    in_maps = []
    for i in range(NCORES):
        in_maps.append({
            "x": np.ascontiguousarray(x[:, NLOC * i:NLOC * (i + 1)]),
            "wupT": wupT, "wconvP": wcvP, "wdnT": wdnT, "jmat": jm,
            "bias": bias, "biasdn2": bdn2,
        })
    res = run_bass_kernel_spmd(nc, in_maps, core_ids=list(range(NCORES)),
                               trace=trace)
    out = np.concatenate([r["out"] for r in res.results], axis=1)
    return out, res


def kernel(**inputs):
    out, _ = run(inputs, trace=False)
    return out
